# revision 4
# baseline (speedup 1.0000x reference)
"""nn_CDA Trainium kernel: dst-sharded GCN message passing on 8 NeuronCores.

v3: fp32 datapath (PCA downstream amplifies GCN-output noise ~26x, so
bf16 aggregation fails the 2e-2 gate; fp32 512B gather descriptors cost
the same as bf16 256B ones - the gather is descriptor-rate-bound).
Structural layout: 3-graph merged programs (one bass program per graph
family) so collectives/dense/scatter phases of different graphs overlap;
block-batched one-hot builds via broadcast-stride DVE ops (norm folded
into gathered rows, binary one-hot).

Device (per 3-layer GCN stack x 3 graphs, one program, SPMD on 8 cores):
  per layer/graph: h' = dinv*(x@W) on row shards, AllGather, edge-gather
  h'[src] via dma_gather (int16 lo/hi regions), scatter-matmul into PSUM,
  rank-1 bias matmul, ReLU+dinv scale.
Host: degree/norm preprocessing, PCA (eigh on CPU jax like the reference),
  projection, l2norm, bilinear decoder.
"""
import numpy as np

P = 128
N_CORES = 8
LO = 32768
NC, ND, FC, K, PPAIR = 60000, 20000, 128, 64, 200000
NPAD_C = 60416   # 8*59*128
NPAD_D = 20480   # 8*20*128


def _pad_to(x, n, val=0):
    if x.shape[0] >= n:
        return x
    w = [(0, n - x.shape[0])] + [(0, 0)] * (x.ndim - 1)
    return np.pad(x, w, constant_values=val)


def _prep_graph(src, dst, ew, n_real, npad):
    sh = npad // N_CORES
    nblk = sh // P
    src = np.concatenate([src.astype(np.int64), np.arange(n_real, dtype=np.int64)])
    dst = np.concatenate([dst.astype(np.int64), np.arange(n_real, dtype=np.int64)])
    ew = np.concatenate([ew.astype(np.float32), np.ones(n_real, np.float32)])
    deg = np.bincount(dst, weights=ew.astype(np.float64), minlength=npad)
    dinv = np.zeros(npad, np.float32)
    nz = deg > 0
    dinv[nz] = (1.0 / np.sqrt(deg[nz])).astype(np.float32)
    recip = np.zeros(npad, np.float32)
    recip[nz] = np.sqrt(deg[nz]).astype(np.float32)
    recip[n_real:] = 0.0
    dinv[n_real:] = 0.0

    order = np.argsort(dst, kind="stable")
    src, dst, ew = src[order], dst[order], ew[order]
    blk_of = dst // P
    starts = np.searchsorted(blk_of, np.arange(N_CORES * nblk + 1))
    lo_mask = src < LO
    nlo = np.zeros(nblk, np.int64)
    nhi = np.zeros(nblk, np.int64)
    for c in range(N_CORES):
        for b in range(nblk):
            s, e = starts[c * nblk + b], starts[c * nblk + b + 1]
            nl = int(lo_mask[s:e].sum())
            nlo[b] = max(nlo[b], (nl + P - 1) // P)
            nhi[b] = max(nhi[b], (e - s - nl + P - 1) // P)
    none = (nlo + nhi) == 0
    nlo[none] = 1
    totch = int((nlo + nhi).sum())

    per_core = []
    for c in range(N_CORES):
        idx = np.zeros((P, totch * 8), np.int16)
        dstloc = np.zeros((P, totch), np.float32)
        normv = np.zeros((P, totch), np.float32)
        ch0 = 0
        for b in range(nblk):
            s, e = starts[c * nblk + b], starts[c * nblk + b + 1]
            m = lo_mask[s:e]
            for part, nch in ((0, int(nlo[b])), (1, int(nhi[b]))):
                if nch == 0:
                    continue
                if part == 0:
                    ss, dd, ww = src[s:e][m], dst[s:e][m], ew[s:e][m]
                else:
                    ss, dd, ww = src[s:e][~m] - LO, dst[s:e][~m], ew[s:e][~m]
                npads = nch * P
                ss = _pad_to(ss, npads)
                ww = _pad_to(ww.astype(np.float32), npads)
                dl = _pad_to((dd % P).astype(np.float32), npads)
                wrapped = ss.reshape(nch * 8, 16).T.astype(np.int16)
                idx[:, ch0 * 8:(ch0 + nch) * 8] = np.tile(wrapped, (8, 1))
                dstloc[:, ch0:ch0 + nch] = dl.reshape(nch, P).T
                normv[:, ch0:ch0 + nch] = ww.reshape(nch, P).T
                ch0 += nch
        per_core.append(dict(idx=idx, dstloc=dstloc, normv=normv))
    return dict(per_core=per_core, nlo=nlo, nhi=nhi, totch=totch,
                dinv=dinv, recip=recip, nblk=nblk, sh=sh)


def _build_program(gs, npad):
    """One bass program running len(gs) 3-layer GCN stacks, interleaved."""
    import concourse.bacc as bacc
    import concourse.mybir as mybir
    import concourse.tile as tile
    from concourse.masks import make_identity

    f32 = mybir.dt.float32
    i16 = mybir.dt.int16
    ng = len(gs)
    sh = gs[0]["sh"]
    nblk = gs[0]["nblk"]
    maxnt = max(int((g["nlo"] + g["nhi"]).max()) for g in gs)

    nc = bacc.Bacc("TRN2", target_bir_lowering=False)
    x_in = nc.declare_dram_parameter("x", [sh, FC], f32, isOutput=False)
    w_in = nc.declare_dram_parameter("w", [3 * ng, FC, FC], f32, isOutput=False)
    b_in = nc.declare_dram_parameter("b", [3 * ng, FC], f32, isOutput=False)
    iota_in = nc.declare_dram_parameter("iota", [P, 1, P], f32, isOutput=False)
    dinv_in, recip_in, idx_in, dst_in, nrm_in = [], [], [], [], []
    outs = []
    h_shards, h_fulls = [], []
    for gi, g in enumerate(gs):
        totch = g["totch"]
        dinv_in.append(nc.declare_dram_parameter(
            f"dinv{gi}", [P, nblk], f32, isOutput=False))
        recip_in.append(nc.declare_dram_parameter(
            f"recip{gi}", [1, sh], f32, isOutput=False))
        idx_in.append(nc.declare_dram_parameter(
            f"idx{gi}", [P, totch * 8], i16, isOutput=False))
        dst_in.append(nc.declare_dram_parameter(
            f"dstloc{gi}", [P, totch], f32, isOutput=False))
        nrm_in.append(nc.declare_dram_parameter(
            f"normv{gi}", [P, totch], f32, isOutput=False))
        outs.append([nc.declare_dram_parameter(
            f"out{gi}_{l}", [sh, FC], f32, isOutput=True) for l in range(3)])
        h_shards.append([nc.dram_tensor(
            f"h_shard{gi}_{l}", [sh, FC], f32) for l in range(3)])
        h_fulls.append([nc.dram_tensor(
            f"h_full{gi}_{l}", [npad, FC], f32, addr_space="Shared")
            for l in range(3)])

    with tile.TileContext(nc) as tc:
        with (
            tc.tile_pool(name="sbuf", bufs=4) as pool,
            tc.tile_pool(name="psum", bufs=2, space="PSUM") as psum_pool,
            tc.tile_pool(name="gpool", bufs=2) as gpool,
            tc.tile_pool(name="mpool", bufs=2) as mpool,
            tc.tile_pool(name="const", bufs=1) as cpool,
        ):
            ident_t = cpool.tile([P, P], f32)
            make_identity(nc, ident_t[:])
            iota_t = cpool.tile([P, 1, P], f32)
            nc.sync.dma_start(out=iota_t[:, :, :], in_=iota_in[:, :, :])
            w_ts, b_ts, dinv_ts, recip_ts = [], [], [], []
            for gi in range(ng):
                for l in range(3):
                    w_t = cpool.tile([FC, FC], f32, tag=f"w{gi}_{l}")
                    nc.sync.dma_start(out=w_t[:], in_=w_in[3 * gi + l])
                    w_ts.append(w_t)
                    b_t = cpool.tile([1, FC], f32, tag=f"b{gi}_{l}")
                    nc.sync.dma_start(out=b_t[:], in_=b_in[3 * gi + l:3 * gi + l + 1, :])
                    b_ts.append(b_t)
                dinv_t = cpool.tile([P, nblk], f32, tag=f"dinv{gi}")
                nc.sync.dma_start(out=dinv_t[:], in_=dinv_in[gi][:, :])
                dinv_ts.append(dinv_t)

            for l in range(3):
                for gi, g in enumerate(gs):
                    x_src = x_in if l == 0 else outs[gi][l - 1]
                    h_shard = h_shards[gi][l]
                    # dense: h' = dinv * (x @ W)
                    for r in range(nblk):
                        xt_ps = psum_pool.tile([P, P], f32, space="PSUM",
                                               tag="xt_ps")
                        x_t = pool.tile([P, P], f32, tag="x_t")
                        nc.sync.dma_start(out=x_t[:],
                                          in_=x_src[r * P:(r + 1) * P, :])
                        nc.tensor.transpose(out=xt_ps[:], in_=x_t[:],
                                            identity=ident_t[:])
                        xt_t = pool.tile([P, P], f32, tag="xt_t")
                        nc.scalar.copy(out=xt_t[:], in_=xt_ps[:])
                        h_ps = psum_pool.tile([P, P], f32, space="PSUM",
                                              tag="h_ps")
                        nc.tensor.matmul(out=h_ps[:], lhsT=xt_t[:],
                                         rhs=w_ts[3 * gi + l][:],
                                         start=True, stop=True)
                        h_t = pool.tile([P, P], f32, tag="h_t")
                        nc.scalar.activation(
                            out=h_t[:], in_=h_ps[:],
                            func=mybir.ActivationFunctionType.Copy,
                            scale=dinv_ts[gi][:, r:r + 1])
                        nc.sync.dma_start(
                            out=h_shard[r * P:(r + 1) * P, :], in_=h_t[:])
                    nc.gpsimd.collective_compute(
                        "AllGather", mybir.AluOpType.bypass,
                        replica_groups=[list(range(N_CORES))],
                        ins=[h_shard[:]], outs=[h_fulls[gi][l][:]])
                for gi, g in enumerate(gs):
                    h_full = h_fulls[gi][l]
                    nlo, nhi = g["nlo"], g["nhi"]
                    lo_end = min(LO, npad)
                    h_lo = h_full[0:lo_end, :]
                    h_hi = h_full[LO:npad, :] if npad > LO else None
                    ch = 0
                    for bk in range(nblk):
                        nl, nh = int(nlo[bk]), int(nhi[bk])
                        ntot = nl + nh
                        g_t = gpool.tile([P, maxnt, P], f32, tag="g_t")
                        idx_t = pool.tile([P, maxnt * 8], i16, tag="idx_t")
                        nc.sync.dma_start(
                            out=idx_t[:, 0:ntot * 8],
                            in_=idx_in[gi][:, ch * 8:(ch + ntot) * 8])
                        meta_d = pool.tile([P, maxnt], f32, tag="meta_d")
                        meta_n = pool.tile([P, maxnt], f32, tag="meta_n")
                        nc.sync.dma_start(out=meta_d[:, 0:ntot],
                                          in_=dst_in[gi][:, ch:ch + ntot])
                        nc.sync.dma_start(out=meta_n[:, 0:ntot],
                                          in_=nrm_in[gi][:, ch:ch + ntot])
                        if nl:
                            nc.gpsimd.dma_gather(
                                g_t[:, 0:nl, :], h_lo, idx_t[:, 0:nl * 8],
                                nl * P, nl * P, P, single_packet=False)
                        if nh:
                            nc.gpsimd.dma_gather(
                                g_t[:, nl:ntot, :], h_hi,
                                idx_t[:, nl * 8:ntot * 8],
                                nh * P, nh * P, P, single_packet=False)
                        # norm into gathered rows; binary one-hot batched
                        nc.vector.tensor_tensor(
                            out=g_t[:, 0:ntot, :], in0=g_t[:, 0:ntot, :],
                            in1=meta_n[:, 0:ntot].broadcast_to([P, ntot, P]),
                            op=mybir.AluOpType.mult)
                        m_t = mpool.tile([P, maxnt, P], f32, tag="m_t")
                        nc.vector.tensor_tensor(
                            out=m_t[:, 0:ntot, :],
                            in0=iota_t[:, :, :].broadcast_to([P, ntot, P]),
                            in1=meta_d[:, 0:ntot].broadcast_to([P, ntot, P]),
                            op=mybir.AluOpType.is_equal)
                        ps = psum_pool.tile([P, P], f32, space="PSUM",
                                            tag="spmm_ps")
                        for cch in range(ntot):
                            nc.tensor.matmul(out=ps[:], lhsT=m_t[:, cch, :],
                                             rhs=g_t[:, cch, :],
                                             start=(cch == 0), stop=False)
                        rsl = pool.tile([1, P], f32, tag="rsl")
                        nc.sync.dma_start(
                            out=rsl[:],
                            in_=recip_in[gi][:, bk * P:(bk + 1) * P])
                        nc.tensor.matmul(
                            out=ps[:], lhsT=rsl[:],
                            rhs=b_ts[3 * gi + l][:], start=False, stop=True)
                        o_t = pool.tile([P, P], f32, tag="o_t")
                        nc.scalar.activation(
                            out=o_t[:], in_=ps[:],
                            func=mybir.ActivationFunctionType.Relu,
                            scale=dinv_ts[gi][:, bk:bk + 1])
                        nc.sync.dma_start(
                            out=outs[gi][l][bk * P:(bk + 1) * P, :], in_=o_t[:])
                        ch += ntot
    nc.finalize()
    return nc, maxnt


def _in_maps_for(gs, x0_full, W9, b9, maxnt):
    sh = gs[0]["sh"]
    nblk = gs[0]["nblk"]
    iota_np = np.ascontiguousarray(np.broadcast_to(
        np.arange(P, dtype=np.float32)[None, None, :], (P, 1, P)))
    in_maps = []
    for c in range(N_CORES):
        m = dict(
            x=np.ascontiguousarray(x0_full[c * sh:(c + 1) * sh]),
            w=W9.astype(np.float32), b=b9.astype(np.float32), iota=iota_np,
        )
        for gi, g in enumerate(gs):
            pc = g["per_core"][c]
            m[f"dinv{gi}"] = np.ascontiguousarray(
                g["dinv"][c * sh:(c + 1) * sh].reshape(nblk, P).T)
            m[f"recip{gi}"] = g["recip"][c * sh:(c + 1) * sh][None, :]
            m[f"idx{gi}"] = pc["idx"]
            m[f"dstloc{gi}"] = pc["dstloc"]
            m[f"normv{gi}"] = pc["normv"]
        in_maps.append(m)
    return in_maps


def _run_family(gs, x0_full, W9, b9, npad):
    """Run 3 stacks (one graph family). Returns [ng, 3, npad, FC]."""
    from concourse.bass_utils import run_bass_kernel_spmd

    nc, maxnt = _build_program(gs, npad)
    in_maps = _in_maps_for(gs, x0_full, W9, b9, maxnt)
    res = run_bass_kernel_spmd(nc, in_maps, core_ids=list(range(N_CORES)))
    out = np.zeros((len(gs), 3, npad, FC), np.float32)
    for gi in range(len(gs)):
        for l in range(3):
            out[gi, l] = np.concatenate(
                [res.results[c][f"out{gi}_{l}"] for c in range(N_CORES)],
                axis=0)
    return out


def _pca_host(Xm, k):
    """PCA matching reference: eigh of covariance on CPU jax (LAPACK)."""
    mu = Xm.mean(axis=0, keepdims=True, dtype=np.float32)
    Xc = (Xm - mu).astype(np.float32)
    cov = (Xc.T @ Xc) / np.float32(Xm.shape[0] - 1)
    try:
        import jax
        with jax.default_device(jax.devices("cpu")[0]):
            _, V = jax.numpy.linalg.eigh(cov)
            V = np.asarray(V)
    except Exception:
        _, V = np.linalg.eigh(cov)
    comp = V[:, ::-1][:, :k]
    return Xc @ comp.astype(np.float32)


def _l2norm(x):
    n = np.sqrt((x.astype(np.float32) ** 2).sum(axis=1, keepdims=True))
    return x / np.maximum(n, 1e-12)


def kernel(**inputs):
    inp = {k: np.asarray(v) for k, v in inputs.items()}
    x_fc = inp["x_fc"].astype(np.float32)
    y_fd = inp["y_fd"].astype(np.float32)
    Wc, bc = inp["Wc"].astype(np.float32), inp["bc"].astype(np.float32)
    Wd, bd = inp["Wd"].astype(np.float32), inp["bd"].astype(np.float32)

    x0c = np.zeros((NPAD_C, FC), np.float32)
    x0c[:NC] = x_fc
    x0d = np.zeros((NPAD_D, FC), np.float32)
    x0d[:ND] = y_fd

    cc_gs = []
    for name in ("cc_g", "cc_c", "cc_sem"):
        e = inp[f"{name}_edges"]
        cc_gs.append(_prep_graph(e[0], e[1], inp[f"{name}_w"], NC, NPAD_C))
    dd_gs = []
    for name in ("dd_g", "dd_c", "dd_dag"):
        e = inp[f"{name}_edges"]
        dd_gs.append(_prep_graph(e[0], e[1], inp[f"{name}_w"], ND, NPAD_D))

    oc = _run_family(cc_gs, x0c, Wc, bc, NPAD_C)
    od = _run_family(dd_gs, x0d, Wd, bd, NPAD_D)

    xs_out = [oc[gi, l][:NC] for gi in range(3) for l in range(3)]
    ys_out = [od[gi, l][:ND] for gi in range(3) for l in range(3)]

    XM = np.concatenate(xs_out, axis=1)
    YD = np.concatenate(ys_out, axis=1)
    XM = _l2norm(_pca_host(XM, K))
    YD = _l2norm(_pca_host(YD, K))
    XM = np.concatenate([XM, inp["Gra_emb_circrna"].astype(np.float32)], axis=1)
    YD = np.concatenate([YD, inp["Gra_emb_dis"].astype(np.float32)], axis=1)

    ci = inp["circ_index"].astype(np.int64)
    di = inp["dis_index"].astype(np.int64)
    c = XM[ci]
    d = YD[di]
    dec_W = inp["dec_W"].astype(np.float32)
    dec_cls = inp["dec_cls"].astype(np.float32)
    basis = np.stack([((c @ dec_W[k_]) * d).sum(axis=1) for k_ in range(2)], axis=1)
    return np.maximum(basis @ dec_cls, 0.0).astype(np.float32)


# revision 6
# speedup vs baseline: 1.3072x; 1.3072x over previous
"""nn_CDA Trainium kernel: dst-sharded GCN message passing on 8 NeuronCores.

v3: fp32 datapath (PCA downstream amplifies GCN-output noise ~26x, so
bf16 aggregation fails the 2e-2 gate; fp32 512B gather descriptors cost
the same as bf16 256B ones - the gather is descriptor-rate-bound).
Structural layout: 3-graph merged programs (one bass program per graph
family) so collectives/dense/scatter phases of different graphs overlap;
block-batched one-hot builds via broadcast-stride DVE ops (norm folded
into gathered rows, binary one-hot).

Device (per 3-layer GCN stack x 3 graphs, one program, SPMD on 8 cores):
  per layer/graph: h' = dinv*(x@W) on row shards, AllGather, edge-gather
  h'[src] via dma_gather (int16 lo/hi regions), scatter-matmul into PSUM,
  rank-1 bias matmul, ReLU+dinv scale.
Host: degree/norm preprocessing, PCA (eigh on CPU jax like the reference),
  projection, l2norm, bilinear decoder.
"""
import numpy as np

P = 128
N_CORES = 8
LO = 32768
NC, ND, FC, K, PPAIR = 60000, 20000, 128, 64, 200000
NPAD_C = 60416   # 8*59*128
NPAD_D = 20480   # 8*20*128


def _pad_to(x, n, val=0):
    if x.shape[0] >= n:
        return x
    w = [(0, n - x.shape[0])] + [(0, 0)] * (x.ndim - 1)
    return np.pad(x, w, constant_values=val)


def _prep_graph(src, dst, ew, n_real, npad):
    sh = npad // N_CORES
    nblk = sh // P
    src = np.concatenate([src.astype(np.int64), np.arange(n_real, dtype=np.int64)])
    dst = np.concatenate([dst.astype(np.int64), np.arange(n_real, dtype=np.int64)])
    ew = np.concatenate([ew.astype(np.float32), np.ones(n_real, np.float32)])
    deg = np.bincount(dst, weights=ew.astype(np.float64), minlength=npad)
    dinv = np.zeros(npad, np.float32)
    nz = deg > 0
    dinv[nz] = (1.0 / np.sqrt(deg[nz])).astype(np.float32)
    recip = np.zeros(npad, np.float32)
    recip[nz] = np.sqrt(deg[nz]).astype(np.float32)
    recip[n_real:] = 0.0
    dinv[n_real:] = 0.0

    order = np.argsort(dst, kind="stable")
    src, dst, ew = src[order], dst[order], ew[order]
    blk_of = dst // P
    starts = np.searchsorted(blk_of, np.arange(N_CORES * nblk + 1))
    lo_mask = src < LO
    nlo = np.zeros(nblk, np.int64)
    nhi = np.zeros(nblk, np.int64)
    for c in range(N_CORES):
        for b in range(nblk):
            s, e = starts[c * nblk + b], starts[c * nblk + b + 1]
            nl = int(lo_mask[s:e].sum())
            nlo[b] = max(nlo[b], (nl + P - 1) // P)
            nhi[b] = max(nhi[b], (e - s - nl + P - 1) // P)
    none = (nlo + nhi) == 0
    nlo[none] = 1
    totch = int((nlo + nhi).sum())

    per_core = []
    for c in range(N_CORES):
        idx = np.zeros((P, totch * 8), np.int16)
        dstloc = np.zeros((P, totch), np.float32)
        normv = np.zeros((P, totch), np.float32)
        ch0 = 0
        for b in range(nblk):
            s, e = starts[c * nblk + b], starts[c * nblk + b + 1]
            m = lo_mask[s:e]
            for part, nch in ((0, int(nlo[b])), (1, int(nhi[b]))):
                if nch == 0:
                    continue
                if part == 0:
                    ss, dd, ww = src[s:e][m], dst[s:e][m], ew[s:e][m]
                else:
                    ss, dd, ww = src[s:e][~m] - LO, dst[s:e][~m], ew[s:e][~m]
                npads = nch * P
                ss = _pad_to(ss, npads)
                ww = _pad_to(ww.astype(np.float32), npads)
                dl = _pad_to((dd % P).astype(np.float32), npads)
                wrapped = ss.reshape(nch * 8, 16).T.astype(np.int16)
                idx[:, ch0 * 8:(ch0 + nch) * 8] = np.tile(wrapped, (8, 1))
                dstloc[:, ch0:ch0 + nch] = dl.reshape(nch, P).T
                normv[:, ch0:ch0 + nch] = ww.reshape(nch, P).T
                ch0 += nch
        per_core.append(dict(idx=idx, dstloc=dstloc, normv=normv))
    return dict(per_core=per_core, nlo=nlo, nhi=nhi, totch=totch,
                dinv=dinv, recip=recip, nblk=nblk, sh=sh)


def _build_program(fams):
    """One bass program running all 3-layer GCN stacks of all families,
    interleaved. fams: list of dicts with keys gs (list of graph preps),
    npad, tag (name prefix)."""
    import concourse.bacc as bacc
    import concourse.mybir as mybir
    import concourse.tile as tile
    from concourse.masks import make_identity

    f32 = mybir.dt.float32
    i16 = mybir.dt.int16
    maxnt = max(int((g["nlo"] + g["nhi"]).max())
                for f in fams for g in f["gs"])

    nc = bacc.Bacc("TRN2", target_bir_lowering=False)
    iota_in = nc.declare_dram_parameter("iota", [P, 1, P], f32, isOutput=False)
    for fam in fams:
        gs, npad, tg = fam["gs"], fam["npad"], fam["tag"]
        ng = len(gs)
        sh = gs[0]["sh"]
        nblk = gs[0]["nblk"]
        fam["x_in"] = nc.declare_dram_parameter(
            f"x{tg}", [sh, FC], f32, isOutput=False)
        fam["w_in"] = nc.declare_dram_parameter(
            f"w{tg}", [3 * ng, FC, FC], f32, isOutput=False)
        fam["b_in"] = nc.declare_dram_parameter(
            f"b{tg}", [3 * ng, FC], f32, isOutput=False)
        fam["dinv_in"], fam["recip_in"] = [], []
        fam["idx_in"], fam["dst_in"], fam["nrm_in"] = [], [], []
        fam["outs"], fam["h_shards"], fam["h_fulls"] = [], [], []
        for gi, g in enumerate(gs):
            totch = g["totch"]
            fam["dinv_in"].append(nc.declare_dram_parameter(
                f"dinv{tg}{gi}", [P, nblk], f32, isOutput=False))
            fam["recip_in"].append(nc.declare_dram_parameter(
                f"recip{tg}{gi}", [1, sh], f32, isOutput=False))
            fam["idx_in"].append(nc.declare_dram_parameter(
                f"idx{tg}{gi}", [P, totch * 8], i16, isOutput=False))
            fam["dst_in"].append(nc.declare_dram_parameter(
                f"dstloc{tg}{gi}", [P, totch], f32, isOutput=False))
            fam["nrm_in"].append(nc.declare_dram_parameter(
                f"normv{tg}{gi}", [P, totch], f32, isOutput=False))
            fam["outs"].append([nc.declare_dram_parameter(
                f"out{tg}{gi}_{l}", [sh, FC], f32, isOutput=True)
                for l in range(3)])
            fam["h_shards"].append([nc.dram_tensor(
                f"h_shard{tg}{gi}_{l}", [sh, FC], f32) for l in range(3)])
            fam["h_fulls"].append([nc.dram_tensor(
                f"h_full{tg}{gi}_{l}", [npad, FC], f32, addr_space="Shared")
                for l in range(3)])

    with tile.TileContext(nc) as tc:
        with (
            tc.tile_pool(name="sbuf", bufs=4) as pool,
            tc.tile_pool(name="psum", bufs=2, space="PSUM") as psum_pool,
            tc.tile_pool(name="gpool", bufs=2) as gpool,
            tc.tile_pool(name="mpool", bufs=2) as mpool,
            tc.tile_pool(name="const", bufs=1) as cpool,
        ):
            ident_t = cpool.tile([P, P], f32)
            make_identity(nc, ident_t[:])
            iota_t = cpool.tile([P, 1, P], f32)
            nc.sync.dma_start(out=iota_t[:, :, :], in_=iota_in[:, :, :])
            for fam in fams:
                tg = fam["tag"]
                fam["w_ts"], fam["b_ts"], fam["dinv_ts"] = [], [], []
                for gi in range(len(fam["gs"])):
                    for l in range(3):
                        w_t = cpool.tile([FC, FC], f32, tag=f"w{tg}{gi}_{l}")
                        nc.sync.dma_start(out=w_t[:],
                                          in_=fam["w_in"][3 * gi + l])
                        fam["w_ts"].append(w_t)
                        b_t = cpool.tile([1, FC], f32, tag=f"b{tg}{gi}_{l}")
                        nc.sync.dma_start(
                            out=b_t[:],
                            in_=fam["b_in"][3 * gi + l:3 * gi + l + 1, :])
                        fam["b_ts"].append(b_t)
                    dinv_t = cpool.tile([P, fam["gs"][0]["nblk"]], f32,
                                        tag=f"dinv{tg}{gi}")
                    nc.sync.dma_start(out=dinv_t[:],
                                      in_=fam["dinv_in"][gi][:, :])
                    fam["dinv_ts"].append(dinv_t)

            for l in range(3):
              for fam in fams:
                gs, npad = fam["gs"], fam["npad"]
                nblk = gs[0]["nblk"]
                w_ts, b_ts, dinv_ts = fam["w_ts"], fam["b_ts"], fam["dinv_ts"]
                recip_in, idx_in = fam["recip_in"], fam["idx_in"]
                dst_in, nrm_in = fam["dst_in"], fam["nrm_in"]
                outs, h_shards, h_fulls = (fam["outs"], fam["h_shards"],
                                           fam["h_fulls"])
                for gi, g in enumerate(gs):
                    x_src = fam["x_in"] if l == 0 else outs[gi][l - 1]
                    h_shard = h_shards[gi][l]
                    # dense: h' = dinv * (x @ W)
                    for r in range(nblk):
                        xt_ps = psum_pool.tile([P, P], f32, space="PSUM",
                                               tag="xt_ps")
                        x_t = pool.tile([P, P], f32, tag="x_t")
                        nc.sync.dma_start(out=x_t[:],
                                          in_=x_src[r * P:(r + 1) * P, :])
                        nc.tensor.transpose(out=xt_ps[:], in_=x_t[:],
                                            identity=ident_t[:])
                        xt_t = pool.tile([P, P], f32, tag="xt_t")
                        nc.scalar.copy(out=xt_t[:], in_=xt_ps[:])
                        h_ps = psum_pool.tile([P, P], f32, space="PSUM",
                                              tag="h_ps")
                        nc.tensor.matmul(out=h_ps[:], lhsT=xt_t[:],
                                         rhs=w_ts[3 * gi + l][:],
                                         start=True, stop=True)
                        h_t = pool.tile([P, P], f32, tag="h_t")
                        nc.scalar.activation(
                            out=h_t[:], in_=h_ps[:],
                            func=mybir.ActivationFunctionType.Copy,
                            scale=dinv_ts[gi][:, r:r + 1])
                        nc.sync.dma_start(
                            out=h_shard[r * P:(r + 1) * P, :], in_=h_t[:])
                    nc.gpsimd.collective_compute(
                        "AllGather", mybir.AluOpType.bypass,
                        replica_groups=[list(range(N_CORES))],
                        ins=[h_shard[:]], outs=[h_fulls[gi][l][:]])
                for gi, g in enumerate(gs):
                    h_full = h_fulls[gi][l]
                    nlo, nhi = g["nlo"], g["nhi"]
                    lo_end = min(LO, npad)
                    h_lo = h_full[0:lo_end, :]
                    h_hi = h_full[LO:npad, :] if npad > LO else None
                    ch = 0
                    for bk in range(nblk):
                        nl, nh = int(nlo[bk]), int(nhi[bk])
                        ntot = nl + nh
                        g_t = gpool.tile([P, maxnt, P], f32, tag="g_t")
                        idx_t = pool.tile([P, maxnt * 8], i16, tag="idx_t")
                        nc.sync.dma_start(
                            out=idx_t[:, 0:ntot * 8],
                            in_=idx_in[gi][:, ch * 8:(ch + ntot) * 8])
                        meta_d = pool.tile([P, maxnt], f32, tag="meta_d")
                        meta_n = pool.tile([P, maxnt], f32, tag="meta_n")
                        nc.sync.dma_start(out=meta_d[:, 0:ntot],
                                          in_=dst_in[gi][:, ch:ch + ntot])
                        nc.sync.dma_start(out=meta_n[:, 0:ntot],
                                          in_=nrm_in[gi][:, ch:ch + ntot])
                        if nl:
                            nc.gpsimd.dma_gather(
                                g_t[:, 0:nl, :], h_lo, idx_t[:, 0:nl * 8],
                                nl * P, nl * P, P, single_packet=False)
                        if nh:
                            nc.gpsimd.dma_gather(
                                g_t[:, nl:ntot, :], h_hi,
                                idx_t[:, nl * 8:ntot * 8],
                                nh * P, nh * P, P, single_packet=False)
                        # norm into gathered rows; binary one-hot batched
                        nc.vector.tensor_tensor(
                            out=g_t[:, 0:ntot, :], in0=g_t[:, 0:ntot, :],
                            in1=meta_n[:, 0:ntot].broadcast_to([P, ntot, P]),
                            op=mybir.AluOpType.mult)
                        m_t = mpool.tile([P, maxnt, P], f32, tag="m_t")
                        nc.vector.tensor_tensor(
                            out=m_t[:, 0:ntot, :],
                            in0=iota_t[:, :, :].broadcast_to([P, ntot, P]),
                            in1=meta_d[:, 0:ntot].broadcast_to([P, ntot, P]),
                            op=mybir.AluOpType.is_equal)
                        ps = psum_pool.tile([P, P], f32, space="PSUM",
                                            tag="spmm_ps")
                        for cch in range(ntot):
                            nc.tensor.matmul(out=ps[:], lhsT=m_t[:, cch, :],
                                             rhs=g_t[:, cch, :],
                                             start=(cch == 0), stop=False)
                        rsl = pool.tile([1, P], f32, tag="rsl")
                        nc.sync.dma_start(
                            out=rsl[:],
                            in_=recip_in[gi][:, bk * P:(bk + 1) * P])
                        nc.tensor.matmul(
                            out=ps[:], lhsT=rsl[:],
                            rhs=b_ts[3 * gi + l][:], start=False, stop=True)
                        o_t = pool.tile([P, P], f32, tag="o_t")
                        nc.scalar.activation(
                            out=o_t[:], in_=ps[:],
                            func=mybir.ActivationFunctionType.Relu,
                            scale=dinv_ts[gi][:, bk:bk + 1])
                        nc.sync.dma_start(
                            out=outs[gi][l][bk * P:(bk + 1) * P, :], in_=o_t[:])
                        ch += ntot
    nc.finalize()
    return nc, maxnt


def _in_maps_for(fams_host, maxnt):
    """fams_host: list of (gs, x0_full, W9, b9, tag)."""
    iota_np = np.ascontiguousarray(np.broadcast_to(
        np.arange(P, dtype=np.float32)[None, None, :], (P, 1, P)))
    in_maps = []
    for c in range(N_CORES):
        m = dict(iota=iota_np)
        for gs, x0_full, W9, b9, tg in fams_host:
            sh = gs[0]["sh"]
            nblk = gs[0]["nblk"]
            m[f"x{tg}"] = np.ascontiguousarray(x0_full[c * sh:(c + 1) * sh])
            m[f"w{tg}"] = W9.astype(np.float32)
            m[f"b{tg}"] = b9.astype(np.float32)
            for gi, g in enumerate(gs):
                pc = g["per_core"][c]
                m[f"dinv{tg}{gi}"] = np.ascontiguousarray(
                    g["dinv"][c * sh:(c + 1) * sh].reshape(nblk, P).T)
                m[f"recip{tg}{gi}"] = g["recip"][c * sh:(c + 1) * sh][None, :]
                m[f"idx{tg}{gi}"] = pc["idx"]
                m[f"dstloc{tg}{gi}"] = pc["dstloc"]
                m[f"normv{tg}{gi}"] = pc["normv"]
        in_maps.append(m)
    return in_maps


def _run_all(fams_host):
    """Run all stacks in one program.

    fams_host: list of (gs, x0_full, W9, b9, tag).
    Returns dict tag -> [ng, 3, npad, FC] fp32.
    """
    from concourse.bass_utils import run_bass_kernel_spmd

    fams = [dict(gs=gs, npad=gs[0]["sh"] * N_CORES, tag=tg)
            for gs, _, _, _, tg in fams_host]
    nc, maxnt = _build_program(fams)
    in_maps = _in_maps_for(fams_host, maxnt)
    res = run_bass_kernel_spmd(nc, in_maps, core_ids=list(range(N_CORES)))
    out = {}
    for gs, _, _, _, tg in fams_host:
        npad = gs[0]["sh"] * N_CORES
        arr = np.zeros((len(gs), 3, npad, FC), np.float32)
        for gi in range(len(gs)):
            for l in range(3):
                arr[gi, l] = np.concatenate(
                    [res.results[c][f"out{tg}{gi}_{l}"]
                     for c in range(N_CORES)], axis=0)
        out[tg] = arr
    return out


def _pca_host(Xm, k):
    """PCA matching reference: eigh of covariance on CPU jax (LAPACK)."""
    mu = Xm.mean(axis=0, keepdims=True, dtype=np.float32)
    Xc = (Xm - mu).astype(np.float32)
    cov = (Xc.T @ Xc) / np.float32(Xm.shape[0] - 1)
    try:
        import jax
        with jax.default_device(jax.devices("cpu")[0]):
            _, V = jax.numpy.linalg.eigh(cov)
            V = np.asarray(V)
    except Exception:
        _, V = np.linalg.eigh(cov)
    comp = V[:, ::-1][:, :k]
    return Xc @ comp.astype(np.float32)


def _l2norm(x):
    n = np.sqrt((x.astype(np.float32) ** 2).sum(axis=1, keepdims=True))
    return x / np.maximum(n, 1e-12)


def kernel(**inputs):
    inp = {k: np.asarray(v) for k, v in inputs.items()}
    x_fc = inp["x_fc"].astype(np.float32)
    y_fd = inp["y_fd"].astype(np.float32)
    Wc, bc = inp["Wc"].astype(np.float32), inp["bc"].astype(np.float32)
    Wd, bd = inp["Wd"].astype(np.float32), inp["bd"].astype(np.float32)

    x0c = np.zeros((NPAD_C, FC), np.float32)
    x0c[:NC] = x_fc
    x0d = np.zeros((NPAD_D, FC), np.float32)
    x0d[:ND] = y_fd

    cc_gs = []
    for name in ("cc_g", "cc_c", "cc_sem"):
        e = inp[f"{name}_edges"]
        cc_gs.append(_prep_graph(e[0], e[1], inp[f"{name}_w"], NC, NPAD_C))
    dd_gs = []
    for name in ("dd_g", "dd_c", "dd_dag"):
        e = inp[f"{name}_edges"]
        dd_gs.append(_prep_graph(e[0], e[1], inp[f"{name}_w"], ND, NPAD_D))

    fams_host = [
        (cc_gs, x0c, Wc, bc, "c"),
        (dd_gs, x0d, Wd, bd, "d"),
    ]
    res = _run_all(fams_host)
    oc, od = res["c"], res["d"]

    xs_out = [oc[gi, l][:NC] for gi in range(3) for l in range(3)]
    ys_out = [od[gi, l][:ND] for gi in range(3) for l in range(3)]

    XM = np.concatenate(xs_out, axis=1)
    YD = np.concatenate(ys_out, axis=1)
    XM = _l2norm(_pca_host(XM, K))
    YD = _l2norm(_pca_host(YD, K))
    XM = np.concatenate([XM, inp["Gra_emb_circrna"].astype(np.float32)], axis=1)
    YD = np.concatenate([YD, inp["Gra_emb_dis"].astype(np.float32)], axis=1)

    ci = inp["circ_index"].astype(np.int64)
    di = inp["dis_index"].astype(np.int64)
    c = XM[ci]
    d = YD[di]
    dec_W = inp["dec_W"].astype(np.float32)
    dec_cls = inp["dec_cls"].astype(np.float32)
    basis = np.stack([((c @ dec_W[k_]) * d).sum(axis=1) for k_ in range(2)], axis=1)
    return np.maximum(basis @ dec_cls, 0.0).astype(np.float32)
